# revision 12
# baseline (speedup 1.0000x reference)
"""Chunked sigmoid MHA on 8 Trainium2 NeuronCores (Bass/Tile).

Problem: out = (sigmoid(scale * (x_q Wq^T)(x_k Wk^T)^T) @ (x_v Wv^T)) @ Wo^T
with B=2, L=S=2048, E=1024, H=16, D=64.

Sharding: (batch, head-group) — core c handles batch b=c//4 and heads
[4g, 4g+4) with g=c%4.  Each core computes its 4 heads' Q/K/V projections
(column slices of Wq/Wk/Wv), full sigmoid attention for those heads, and a
partial output projection (row slice of Wo^T); the host sums the 4 partial
outputs per batch.

The kernel is ACT-bound: 128 sigmoid instructions of [128, 1024] are
~143.7 us of scalar-engine time, while true PE stream time is ~117 us
(score/attn-out matmul pairs run concurrently via tile_position row/col
tiling).  The design therefore optimizes ACT occupancy:
  - all host tensors are pre-linearized so each SBUF tile is one
    contiguous DMA (2KB runs per partition), spread over all 5 engine
    queues; critical wk/wq/xk0/xq0 land first -> first sigmoid ~16us
  - the sigmoid ACT table is preloaded at t~0
  - lc0 interleaves per-s-group [scores | k-proj(next) | v-proj | attn-out]
    so scores (which feed ACT) are never queued behind arrival-gated
    projection work; q projections and output projections are deferred
    to lc1-3 where the PE is underloaded
  - output is stored bf16 (halves store traffic), host accumulates f32
"""

import ml_dtypes
import numpy as np

import concourse.bass as bass
import concourse.mybir as mybir
import concourse.tile as tile
from concourse import bass_utils
from concourse.vector_clock import ScopedClock

F32 = mybir.dt.float32
BF16 = mybir.dt.bfloat16
AF = mybir.ActivationFunctionType

E = 1024          # embed dim
L = 2048          # sequence length (queries == keys)
DH = 256          # per-core projection dim (4 heads x 64)
EC = E // 128     # 8 E-chunks of 128
LC = L // 512     # 4 L-chunks of 512
ST = L // 128     # 16 S-tiles of 128
SCALE = 64 ** -0.5  # 0.125, applied inside the sigmoid activation

N_CORES = 8


class SplitDrainTileContext(tile.TileContext):
    """This walrus build rejects >1 sync wait on the SP CTRL (Drain)
    instruction, and Tile's end-of-kernel drain waits on every used proc.
    Split the waits across a chain of single-wait drains."""

    DRAIN_WAIT_CAP = 1

    def _drain_and_barrier(self, tick_clock, wait_clock):
        nc = self.nc
        drain_inst = nc.sync.drain()
        wait_clock.add_sem_waits(
            drain_inst.ins, ScopedClock({None: tick_clock.global_clock})
        )
        si = drain_inst.ins.sync_info
        waits = list(si.on_wait) if si is not None else []
        if len(waits) > self.DRAIN_WAIT_CAP:
            si.on_wait = waits[: self.DRAIN_WAIT_CAP]
            for i in range(self.DRAIN_WAIT_CAP, len(waits), self.DRAIN_WAIT_CAP):
                extra = nc.sync.drain()
                esi = extra.ins.sync_info
                if esi is None:
                    esi = mybir.SyncInfo(on_wait=[], on_update=[])
                esi.on_wait = waits[i : i + self.DRAIN_WAIT_CAP]
                extra.ins.sync_info = esi
        nc.all_engine_barrier()
        assert self.sems is not None
        popped = nc._tile_sem_poison_stack.pop()
        assert popped is self._sem_poison
        nc.clear_and_free_semaphores(list(self.sems.allocated().values()))
        nc.all_engine_barrier()


def build_nc() -> bass.Bass:
    nc = bass.Bass("TRN2", target_bir_lowering=False, debug=False)

    # Host-linearized layouts (see _prep_in_maps):
    #   x*_lin [128, 16384]: col (c*8+e)*512 + l  =  xT[e*128+p, c*512+l]
    #   w*_lin [128, 2048]:  col e*256 + m        =  w*T[e*128+p, m]
    #   wo_lin [128, 2048]:  col m*1024 + eo      =  wo[m*128+p, eo]
    xq = nc.dram_tensor("xq_lin", [128, 16384], BF16, kind="ExternalInput").ap()
    xk = nc.dram_tensor("xk_lin", [128, 16384], BF16, kind="ExternalInput").ap()
    xv = nc.dram_tensor("xv_lin", [128, 16384], BF16, kind="ExternalInput").ap()
    wq = nc.dram_tensor("wq_lin", [128, 2048], BF16, kind="ExternalInput").ap()
    wk = nc.dram_tensor("wk_lin", [128, 2048], BF16, kind="ExternalInput").ap()
    wv = nc.dram_tensor("wv_lin", [128, 2048], BF16, kind="ExternalInput").ap()
    wo = nc.dram_tensor("wo_lin", [128, 2048], BF16, kind="ExternalInput").ap()
    out = nc.dram_tensor("out", [L, E], BF16, kind="ExternalOutput").ap()

    with SplitDrainTileContext(nc) as tc:
        body(tc, xq, xk, xv, wq, wk, wv, wo, out)
    _split_waits(nc)
    return nc


def _split_waits(nc, cap=1):
    """This walrus build rejects instructions carrying more than one sync
    wait.  Hoist excess waits onto same-engine NoOps inserted immediately
    before the instruction (engine program order enforces them first)."""
    ctr = 0
    for f in nc.m.functions:
        for bb in f.blocks:
            new = []
            for inst in bb.instructions:
                si = inst.sync_info
                waits = list(si.on_wait) if si is not None else []
                if len(waits) > cap:
                    for i in range(cap, len(waits), cap):
                        ctr += 1
                        nop = mybir.InstNoOp(name=f"I-waitnop-{ctr}")
                        nop.engine = inst.engine
                        nop.sync_info = mybir.SyncInfo(
                            on_wait=waits[i : i + cap], on_update=[]
                        )
                        nc.register_instruction(nop)
                        new.append(nop)
                    si.on_wait = waits[:cap]
                new.append(inst)
            bb.instructions = new
    return ctr


def body(tc, xq, xk, xv, wq, wk, wv, wo, out):
    nc = tc.nc

    # ---- persistent SBUF tensors -------------------------------------
    persist = tc.alloc_tile_pool(name="persist", bufs=1)

    def ptile(name, shape):
        return persist.tile(shape, BF16, tag=name, name=name)

    # weights, E-chunk-major: w*_sb[:, e*256+m] = w*T[e*128+p, m]
    wq_sb = ptile("wq_sb", [128, 2048])
    wk_sb = ptile("wk_sb", [128, 2048])
    wv_sb = ptile("wv_sb", [128, 2048])
    # wo, m-chunk-major: wo_sb[:, m*1024+e] = wo[m*128+p, e]
    wo_sb = ptile("wo_sb", [128, 2 * E])
    # projected tensors: qT/kT [dh, L] stored Mt-major; v natural [S, dh]
    # stored St-major; oT [dh, L] stored m-chunk-major
    qT_sb = ptile("qT_sb", [128, 2 * L])
    kT_sb = ptile("kT_sb", [128, 2 * L])
    v_sb = persist.tile([128, ST * DH], BF16, tag="v_sb", name="v_sb")
    oT_sb = ptile("oT_sb", [128, 2 * L])
    scratch = persist.tile([128, 512], BF16, tag="scratch", name="scratch")
    act_warm = persist.tile([128, 8], BF16, tag="act_warm", name="act_warm")

    # sc bufs=24 gives ACT/attn-out a 12-step elastic window so late v
    # arrivals (v-chunks are the lowest-priority DMAs) never stall the
    # sigmoid chain
    sc_pool = tc.alloc_tile_pool(name="sc", bufs=24)
    ou_pool = tc.alloc_tile_pool(name="ou", bufs=3)
    xin = tc.alloc_tile_pool(name="xin", bufs=48)
    ps_proj = tc.alloc_tile_pool(name="ps_proj", bufs=2, space="PSUM")
    ps_sc = tc.alloc_tile_pool(name="ps_sc", bufs=2, space="PSUM")
    ps_o = tc.alloc_tile_pool(name="ps_o", bufs=2, space="PSUM")

    # ---- x tiles + the DMA program -----------------------------------
    # xtiles[(nm, c, j)] is a [128, 1024] tile holding e-chunks 2j, 2j+1
    # of L-chunk c — one contiguous slice of the host-linearized x*_lin,
    # so each tile is a single DMA with 2KB runs per partition.
    xsrc = {"q": xq, "k": xk, "v": xv}
    xtiles = {}
    for nm in ("k", "q", "v"):
        for c in range(LC):
            for j in range(4):
                xtiles[(nm, c, j)] = xin.tile(
                    [128, 1024], BF16, tag="xin", name=f"x{nm}{c}_{j}"
                )

    def xd(nm, c, j):
        def go(eng):
            col = (c * 4 + j) * 1024
            eng.dma_start(xtiles[(nm, c, j)][:], xsrc[nm][:, col : col + 1024])
        return go

    def wd(wsb, wsrc, g):
        def go(eng):
            sl = slice(g * 1024, (g + 1) * 1024)
            eng.dma_start(wsb[:, sl], wsrc[:, sl])
        return go

    # Only 3 DMA queues exist: sync + scalar (HWDGE) and gpsimd (SWDGE).
    # Measured rates: gpsimd ~175 GB/s (fastest), scalar ~98, sync
    # ~100-135 with a ~6us slow start — so gpsimd carries the critical
    # wave (wk, xk0, xq0) and all k-chunks; scalar carries wq/v0/q1/wo;
    # sync gets second copies and the late chunks.  k-chunks lead
    # v-chunks: scores pace ACT while attn-out may lag behind the sc
    # tile buffer.  "act" preloads the sigmoid ACT table early.  The
    # memset for the warmup tiles runs on vector so the gpsimd queue
    # starts issuing immediately.
    nc.vector.memset(scratch[:], 0.0)

    def act_preload(_):
        nc.scalar.activation(act_warm[:], scratch[:, :8], AF.Sigmoid, scale=SCALE)

    dma_program = {
        "gpsimd": [
            wd(wk_sb, wk, 0), xd("k", 0, 0), xd("k", 0, 1), xd("k", 0, 2),
            xd("k", 0, 3), xd("q", 0, 0), xd("q", 0, 1),
            xd("k", 1, 0), xd("k", 1, 1), xd("k", 1, 2), xd("k", 2, 0),
            xd("k", 2, 1), xd("k", 2, 2), xd("k", 3, 0), xd("k", 3, 1),
            xd("v", 0, 0), xd("v", 0, 1), xd("k", 3, 2),
            xd("v", 1, 0), xd("v", 1, 1), xd("v", 2, 0), xd("v", 2, 1),
            xd("v", 3, 0), xd("v", 3, 1), xd("q", 2, 0), xd("q", 2, 1),
            xd("q", 3, 0), xd("q", 3, 1),
        ],
        "scalar": [
            wd(wk_sb, wk, 1), wd(wq_sb, wq, 0), wd(wq_sb, wq, 1),
            xd("q", 0, 3), act_preload,
            xd("v", 0, 2), xd("v", 0, 3), xd("q", 1, 0), xd("q", 1, 1),
            xd("q", 1, 2), xd("q", 1, 3), wd(wo_sb, wo, 0), wd(wo_sb, wo, 1),
        ],
        "sync": [
            xd("q", 0, 2), wd(wv_sb, wv, 0), wd(wv_sb, wv, 1),
            xd("k", 1, 3), xd("k", 2, 3), xd("k", 3, 3),
            xd("v", 1, 2), xd("v", 1, 3), xd("v", 2, 2), xd("v", 2, 3),
            xd("v", 3, 2), xd("v", 3, 3), xd("q", 2, 2), xd("q", 2, 3),
            xd("q", 3, 2), xd("q", 3, 3),
        ],
    }
    # The two HWDGE queues (scalar, sync) share one 8-deep ring of
    # flow-control semaphores assigned in EMISSION order: emitting one
    # queue's whole program first makes the other queue's issues wait on
    # the first queue's last transfers.  Interleave emission round-robin
    # so the ring distance stays short in time on both queues.
    progs = [
        (getattr(nc, n), list(p)) for n, p in dma_program.items()
    ]
    i = 0
    while any(p for _, p in progs):
        eng, p = progs[i % len(progs)]
        if p:
            p.pop(0)(eng)
        i += 1

    # Warm the PE (HAM clock gate) with scratch matmuls while the first
    # DMAs are in flight.
    wu_ps = ps_sc.tile([128, 1024], F32, tag="ps_sc", name="warmup_ps")
    for i in range(10):
        nc.tensor.matmul(
            wu_ps[:, :512], lhsT=scratch[:, :128], rhs=scratch[:],
            start=(i == 0), stop=(i == 9),
        )

    # ---- projection emitters -----------------------------------------
    def kq_units(nm, c):
        """8 units (one per e-chunk) of the k-/q-projection of L-chunk c."""
        wsb, dst = (wk_sb, kT_sb) if nm == "k" else (wq_sb, qT_sb)
        acc = []

        def unit(e):
            if e == 0:
                acc.extend(
                    ps_proj.tile([128, 512], F32, tag="ps_proj", name=f"{nm}{c}_{mt}")
                    for mt in range(2)
                )
            xt = xtiles[(nm, c, e // 2)][:, (e % 2) * 512 : (e % 2) * 512 + 512]
            for mt in range(2):
                nc.tensor.matmul(
                    acc[mt][:],
                    lhsT=wsb[:, e * DH + mt * 128 : e * DH + (mt + 1) * 128],
                    rhs=xt,
                    start=(e == 0),
                    stop=(e == EC - 1),
                )
            if e == EC - 1:
                for mt in range(2):
                    nc.vector.tensor_copy(
                        dst[:, mt * L + c * 512 : mt * L + (c + 1) * 512],
                        acc[mt][:],
                    )

        for e in range(EC):
            yield lambda e=e: unit(e)

    def v_units(c):
        """8 units ((st4, e-half)) of the v-projection of L-chunk c."""
        for st4 in range(4):
            box = {}

            def unit(st4, eh, box):
                st = c * 4 + st4
                if eh == 0:
                    box["acc"] = ps_proj.tile(
                        [128, DH], F32, tag="ps_proj", name=f"vacc{st}"
                    )
                for e in range(eh * 4, eh * 4 + 4):
                    nc.tensor.matmul(
                        box["acc"][:],
                        lhsT=xtiles[("v", c, e // 2)][
                            :, (e % 2) * 512 + st4 * 128 : (e % 2) * 512 + (st4 + 1) * 128
                        ],
                        rhs=wv_sb[:, e * DH : (e + 1) * DH],
                        start=(e == 0),
                        stop=(e == EC - 1),
                    )
                if eh == 1:
                    nc.vector.tensor_copy(v_sb[:, st * DH : (st + 1) * DH], box["acc"][:])

            for eh in range(2):
                yield lambda st4=st4, eh=eh, box=box: unit(st4, eh, box)

    def outproj_units(lc):
        """4 units (one per l-tile): out[lg:lg+128, :] = oT.T @ wo, cast
        to bf16, one 256KB row-contiguous DMA store.  The last l-chunk
        also stores via the scalar queue — ACT is done by then and three
        queues shorten the tail drain."""
        engs = (
            [nc.sync, nc.gpsimd, nc.scalar, nc.sync]
            if lc == LC - 1
            else [nc.sync, nc.gpsimd, nc.sync, nc.gpsimd]
        )
        last = lc == LC - 1
        for lt in range(4):
            def unit(lt=lt):
                lg = lc * 512 + lt * 128
                ot = ou_pool.tile([128, E], BF16, tag="ou", name=f"ot{lc}_{lt}")
                for ec in range(2):
                    # the tail l-chunk spreads psum across ps_proj+ps_sc
                    # (idle by then) and casts on vector+gpsimd so its 16
                    # matmuls run back-to-back instead of ping-ponging on
                    # two slots behind each cast
                    pool = ps_sc if (last and ec == 1) else ps_proj
                    ps = pool.tile(
                        [128, 512], F32, tag=pool is ps_sc and "ps_sc" or "ps_proj",
                        name=f"ops{lc}_{lt}_{ec}",
                    )
                    for m in range(2):
                        nc.tensor.matmul(
                            ps[:],
                            lhsT=oT_sb[:, m * L + lg : m * L + lg + 128],
                            rhs=wo_sb[:, m * E + ec * 512 : m * E + (ec + 1) * 512],
                            start=(m == 0),
                            stop=(m == 1),
                        )
                    # gpsimd can't read PSUM; scalar (ACT) can and has
                    # finished its sigmoids by the tail
                    if last and (lt + ec) % 2:
                        nc.scalar.copy(ot[:, ec * 512 : (ec + 1) * 512], ps[:])
                    else:
                        nc.vector.tensor_copy(ot[:, ec * 512 : (ec + 1) * 512], ps[:])
                engs[lt].dma_start(out[lg : lg + 128, :], ot[:])
            yield unit

    # ---- attention emitters ------------------------------------------
    sc_tiles = {}

    def emit_scores(lc, st):
        for pair in range(2):
            ps = ps_sc.tile([128, 1024], F32, tag="ps_sc", name=f"scps{lc}_{st}_{pair}")
            for sub in range(2):
                nc.tensor.matmul(
                    ps[:, sub * 512 : (sub + 1) * 512],
                    lhsT=kT_sb[
                        sub * 64 : (sub + 1) * 64,
                        pair * L + st * 128 : pair * L + (st + 1) * 128,
                    ],
                    rhs=qT_sb[
                        sub * 64 : (sub + 1) * 64,
                        pair * L + lc * 512 : pair * L + (lc + 1) * 512,
                    ],
                    start=True,
                    stop=True,
                    tile_position=(sub * 64, 0),
                )
            sc = sc_pool.tile([128, 1024], BF16, tag="sc", name=f"sc{lc}_{st}_{pair}")
            nc.scalar.activation(sc[:], ps[:], AF.Sigmoid, scale=SCALE)
            sc_tiles[(st, pair)] = sc

    def emit_attnout(lc, st, o_acc):
        for pair in range(2):
            for sub in range(2):
                h = pair * 2 + sub
                nc.tensor.matmul(
                    o_acc[pair][sub * 64 : (sub + 1) * 64, :],
                    lhsT=v_sb[:, st * DH + h * 64 : st * DH + (h + 1) * 64],
                    rhs=sc_tiles[(st, pair)][:, sub * 512 : (sub + 1) * 512],
                    start=(st == 0),
                    stop=(st == ST - 1),
                    tile_position=(0, sub * 64),
                    # Sim's psum-group bookkeeping mis-addresses
                    # partition-offset groups; has_written is per-element
                    # on HW and the two halves are disjoint.
                    skip_group_check=True,
                )

    filler = []          # queue of pending closures (lc1-3 only)

    def pop_filler(n):
        for _ in range(min(n, len(filler))):
            filler.pop(0)()

    def flush_filler():
        while filler:
            filler.pop(0)()

    # ---- lc0: fully explicit schedule --------------------------------
    # k0/q0 inline (their DMAs land first); per s-group the PE queue is
    # [scores x4 | k(next) | v(sg-1) | attn-out(sg-1) x4]: scores (which
    # pace ACT) are never queued behind arrival-gated v-projections —
    # attn-out runs one s-group behind, inside the sc tile buffer's
    # elastic window, and catches up.
    for u in kq_units("k", 0):
        u()
    for u in kq_units("q", 0):
        u()

    o_acc0 = [
        ps_o.tile([128, 512], F32, tag="ps_o", name=f"oacc0_{p}") for p in range(2)
    ]
    for sg in range(4):
        for st4 in range(4):
            emit_scores(0, sg * 4 + st4)
        if sg < 3:
            for u in kq_units("k", sg + 1):
                u()
        else:
            for u in kq_units("q", 1):
                u()
        if sg >= 1:
            for u in v_units(sg - 1):
                u()
            for st in range((sg - 1) * 4, sg * 4):
                emit_attnout(0, st, o_acc0)

    # ---- lc1 prefix: drain lc0's deferred tail -----------------------
    # v3 + attn-out st12-15 + the lc0 oT copy interleave with lc1's
    # first scores so ACT rolls straight across the l-chunk boundary.
    for u in v_units(3):
        u()
    o_acc1 = [
        ps_o.tile([128, 512], F32, tag="ps_o", name="oacc1_%d" % p) for p in range(2)
    ]
    emit_scores(1, 0)
    emit_scores(1, 1)
    for st in range(12, 16):
        emit_attnout(0, st, o_acc0)
    for pair in range(2):
        nc.vector.tensor_copy(
            oT_sb[:, pair * L + 0 * 512 : pair * L + 1 * 512], o_acc0[pair][:]
        )

    # ---- lc1-3: ACT-bound steady state --------------------------------
    # filler per lc: q(lc+1) projection then outproj(lc-1); popped only
    # from st>=4 so outproj never queues ahead of a not-yet-emitted oT
    # copy or stalls the first scores of the l-chunk.
    filler.extend(kq_units("q", 2))
    filler.extend(outproj_units(0))
    for lc in range(1, LC):
        if lc == 1:
            o_acc = o_acc1
        else:
            o_acc = [
                ps_o.tile([128, 512], F32, tag="ps_o", name=f"oacc{lc}_{p}")
                for p in range(2)
            ]
        for st in range(16):
            if not (lc == 1 and st < 2):
                emit_scores(lc, st)
            emit_attnout(lc, st, o_acc)
            if st >= 4:
                pop_filler(1)
        flush_filler()
        for pair in range(2):
            nc.vector.tensor_copy(
                oT_sb[:, pair * L + lc * 512 : pair * L + (lc + 1) * 512],
                o_acc[pair][:],
            )
        if lc == 1:
            filler.extend(kq_units("q", 3))
            filler.extend(outproj_units(1))
        elif lc == 2:
            filler.extend(outproj_units(2))
        else:
            filler.extend(outproj_units(3))
    flush_filler()

    # release pools in reverse allocation (stack) order
    for pool in (ps_o, ps_sc, ps_proj, xin, ou_pool, sc_pool, persist):
        pool.release()


_NC_CACHE = None


def _get_nc():
    global _NC_CACHE
    if _NC_CACHE is None:
        _NC_CACHE = build_nc()
    return _NC_CACHE


def _lin_x(xT):
    """[1024, 2048] -> [128, 16384] with col (c*8+e)*512+l = xT[e*128+p, c*512+l]."""
    return np.ascontiguousarray(
        xT.reshape(8, 128, 4, 512).transpose(1, 2, 0, 3).reshape(128, 16384)
    )


def _prep_in_maps(query, key, value, Wq, Wk, Wv, Wo):
    B = query.shape[0]
    bf = ml_dtypes.bfloat16
    xlin = {}
    for b in range(B):
        xlin[("q", b)] = _lin_x(query[b].T.astype(bf))
        xlin[("k", b)] = _lin_x(key[b].T.astype(bf))
        xlin[("v", b)] = _lin_x(value[b].T.astype(bf))
    in_maps = []
    for c in range(N_CORES):
        b, g = c // 4, c % 4
        hs = slice(g * DH, (g + 1) * DH)

        def wlin(W):
            wT = np.ascontiguousarray(W[hs, :].T).astype(bf)   # [1024, 256]
            return np.ascontiguousarray(
                wT.reshape(8, 128, 256).transpose(1, 0, 2).reshape(128, 2048)
            )

        woT = np.ascontiguousarray(Wo[:, hs].T).astype(bf)     # [256, 1024]
        wo_l = np.ascontiguousarray(
            woT.reshape(2, 128, 1024).transpose(1, 0, 2).reshape(128, 2048)
        )
        in_maps.append(
            {
                "xq_lin": xlin[("q", b)],
                "xk_lin": xlin[("k", b)],
                "xv_lin": xlin[("v", b)],
                "wq_lin": wlin(Wq),
                "wk_lin": wlin(Wk),
                "wv_lin": wlin(Wv),
                "wo_lin": wo_l,
            }
        )
    return in_maps


LAST_RESULTS = None


def run_sharded(query, key, value, Wq, Wk, Wv, Wo, trace=False, tmpdir=None):
    global LAST_RESULTS
    if trace:
        # Shim the missing antenv.axon_hooks so NTFF tracing works under axon.
        import sys
        import types

        try:
            import antenv.axon_hooks  # noqa: F401
        except ImportError:
            from trn_agent_boot.trn_boot import _ntff_profile_via_ctypes

            _mod = types.ModuleType("antenv.axon_hooks")
            _hook = _ntff_profile_via_ctypes("/opt/axon/libaxon_pjrt.so")
            _mod.get_axon_ntff_profile_hook = lambda: _hook
            sys.modules["antenv.axon_hooks"] = _mod
        bass_utils.upload_artifacts = lambda tmpdir: tmpdir

    nc = _get_nc()
    in_maps = _prep_in_maps(query, key, value, Wq, Wk, Wv, Wo)
    res = bass_utils.run_bass_kernel_spmd(
        nc, in_maps, core_ids=list(range(N_CORES)), trace=trace, tmpdir=tmpdir
    )
    LAST_RESULTS = res
    B = query.shape[0]
    full = np.zeros((B, L, E), dtype=np.float32)
    for c in range(N_CORES):
        full[c // 4] += np.asarray(res.results[c]["out"], dtype=np.float32)
    return full


def kernel(query, key, value, Wq, Wk, Wv, Wo):
    return run_sharded(query, key, value, Wq, Wk, Wv, Wo, trace=False)


# revision 13
# speedup vs baseline: 1.0373x; 1.0373x over previous
"""Chunked sigmoid MHA on 8 Trainium2 NeuronCores (Bass/Tile).

Problem: out = (sigmoid(scale * (x_q Wq^T)(x_k Wk^T)^T) @ (x_v Wv^T)) @ Wo^T
with B=2, L=S=2048, E=1024, H=16, D=64.

Sharding: (batch, head-group) — core c handles batch b=c//4 and heads
[4g, 4g+4) with g=c%4.  Each core computes its 4 heads' Q/K/V projections
(column slices of Wq/Wk/Wv), full sigmoid attention for those heads, and a
partial output projection (row slice of Wo^T); the host sums the 4 partial
outputs per batch.

The kernel is ACT-bound: 128 sigmoid instructions of [128, 1024] are
~143.7 us of scalar-engine time, while true PE stream time is ~117 us
(score/attn-out matmul pairs run concurrently via tile_position row/col
tiling).  The design therefore optimizes ACT occupancy:
  - all host tensors are pre-linearized so each SBUF tile is one
    contiguous DMA (2KB runs per partition), spread over all 5 engine
    queues; critical wk/wq/xk0/xq0 land first -> first sigmoid ~16us
  - the sigmoid ACT table is preloaded at t~0
  - lc0 interleaves per-s-group [scores | k-proj(next) | v-proj | attn-out]
    so scores (which feed ACT) are never queued behind arrival-gated
    projection work; q projections and output projections are deferred
    to lc1-3 where the PE is underloaded
  - output is stored bf16 (halves store traffic), host accumulates f32
"""

import ml_dtypes
import numpy as np

import concourse.bass as bass
import concourse.mybir as mybir
import concourse.tile as tile
from concourse import bass_utils
from concourse.vector_clock import ScopedClock

F32 = mybir.dt.float32
BF16 = mybir.dt.bfloat16
AF = mybir.ActivationFunctionType

E = 1024          # embed dim
L = 2048          # sequence length (queries == keys)
DH = 256          # per-core projection dim (4 heads x 64)
EC = E // 128     # 8 E-chunks of 128
LC = L // 512     # 4 L-chunks of 512
ST = L // 128     # 16 S-tiles of 128
SCALE = 64 ** -0.5  # 0.125, applied inside the sigmoid activation

N_CORES = 8


class SplitDrainTileContext(tile.TileContext):
    """This walrus build rejects >1 sync wait on the SP CTRL (Drain)
    instruction, and Tile's end-of-kernel drain waits on every used proc.
    Split the waits across a chain of single-wait drains."""

    DRAIN_WAIT_CAP = 1

    def _drain_and_barrier(self, tick_clock, wait_clock):
        nc = self.nc
        drain_inst = nc.sync.drain()
        wait_clock.add_sem_waits(
            drain_inst.ins, ScopedClock({None: tick_clock.global_clock})
        )
        si = drain_inst.ins.sync_info
        waits = list(si.on_wait) if si is not None else []
        if len(waits) > self.DRAIN_WAIT_CAP:
            si.on_wait = waits[: self.DRAIN_WAIT_CAP]
            for i in range(self.DRAIN_WAIT_CAP, len(waits), self.DRAIN_WAIT_CAP):
                extra = nc.sync.drain()
                esi = extra.ins.sync_info
                if esi is None:
                    esi = mybir.SyncInfo(on_wait=[], on_update=[])
                esi.on_wait = waits[i : i + self.DRAIN_WAIT_CAP]
                extra.ins.sync_info = esi
        nc.all_engine_barrier()
        assert self.sems is not None
        popped = nc._tile_sem_poison_stack.pop()
        assert popped is self._sem_poison
        nc.clear_and_free_semaphores(list(self.sems.allocated().values()))
        nc.all_engine_barrier()


def build_nc() -> bass.Bass:
    nc = bass.Bass("TRN2", target_bir_lowering=False, debug=False)

    # Host-linearized layouts (see _prep_in_maps):
    #   x*_lin [128, 16384]: col (c*8+e)*512 + l  =  xT[e*128+p, c*512+l]
    #   w*_lin [128, 2048]:  col e*256 + m        =  w*T[e*128+p, m]
    #   wo_lin [128, 2048]:  col m*1024 + eo      =  wo[m*128+p, eo]
    xq = nc.dram_tensor("xq_lin", [128, 16384], BF16, kind="ExternalInput").ap()
    xk = nc.dram_tensor("xk_lin", [128, 16384], BF16, kind="ExternalInput").ap()
    xv = nc.dram_tensor("xv_lin", [128, 16384], BF16, kind="ExternalInput").ap()
    wq = nc.dram_tensor("wq_lin", [128, 2048], BF16, kind="ExternalInput").ap()
    wk = nc.dram_tensor("wk_lin", [128, 2048], BF16, kind="ExternalInput").ap()
    wv = nc.dram_tensor("wv_lin", [128, 2048], BF16, kind="ExternalInput").ap()
    wo = nc.dram_tensor("wo_lin", [128, 2048], BF16, kind="ExternalInput").ap()
    out = nc.dram_tensor("out", [L, E], BF16, kind="ExternalOutput").ap()

    with SplitDrainTileContext(nc) as tc:
        body(tc, xq, xk, xv, wq, wk, wv, wo, out)
    _split_waits(nc)
    return nc


def _split_waits(nc, cap=1):
    """This walrus build rejects instructions carrying more than one sync
    wait.  Hoist excess waits onto same-engine NoOps inserted immediately
    before the instruction (engine program order enforces them first)."""
    ctr = 0
    for f in nc.m.functions:
        for bb in f.blocks:
            new = []
            for inst in bb.instructions:
                si = inst.sync_info
                waits = list(si.on_wait) if si is not None else []
                if len(waits) > cap:
                    for i in range(cap, len(waits), cap):
                        ctr += 1
                        nop = mybir.InstNoOp(name=f"I-waitnop-{ctr}")
                        nop.engine = inst.engine
                        nop.sync_info = mybir.SyncInfo(
                            on_wait=waits[i : i + cap], on_update=[]
                        )
                        nc.register_instruction(nop)
                        new.append(nop)
                    si.on_wait = waits[:cap]
                new.append(inst)
            bb.instructions = new
    return ctr


def body(tc, xq, xk, xv, wq, wk, wv, wo, out):
    nc = tc.nc

    # ---- persistent SBUF tensors -------------------------------------
    persist = tc.alloc_tile_pool(name="persist", bufs=1)

    def ptile(name, shape):
        return persist.tile(shape, BF16, tag=name, name=name)

    # weights, E-chunk-major: w*_sb[:, e*256+m] = w*T[e*128+p, m]
    wq_sb = ptile("wq_sb", [128, 2048])
    wk_sb = ptile("wk_sb", [128, 2048])
    wv_sb = ptile("wv_sb", [128, 2048])
    # wo, m-chunk-major: wo_sb[:, m*1024+e] = wo[m*128+p, e]
    wo_sb = ptile("wo_sb", [128, 2 * E])
    # projected tensors: qT/kT [dh, L] stored Mt-major; v natural [S, dh]
    # stored St-major; oT [dh, L] stored m-chunk-major
    qT_sb = ptile("qT_sb", [128, 2 * L])
    kT_sb = ptile("kT_sb", [128, 2 * L])
    v_sb = persist.tile([128, ST * DH], BF16, tag="v_sb", name="v_sb")
    oT_sb = ptile("oT_sb", [128, 2 * L])
    scratch = persist.tile([128, 512], BF16, tag="scratch", name="scratch")
    act_warm = persist.tile([128, 8], BF16, tag="act_warm", name="act_warm")

    # sc bufs=24 gives ACT/attn-out a 12-step elastic window so late v
    # arrivals (v-chunks are the lowest-priority DMAs) never stall the
    # sigmoid chain
    sc_pool = tc.alloc_tile_pool(name="sc", bufs=24)
    ou_pool = tc.alloc_tile_pool(name="ou", bufs=3)
    xin = tc.alloc_tile_pool(name="xin", bufs=48)
    ps_proj = tc.alloc_tile_pool(name="ps_proj", bufs=2, space="PSUM")
    ps_sc = tc.alloc_tile_pool(name="ps_sc", bufs=2, space="PSUM")
    ps_o = tc.alloc_tile_pool(name="ps_o", bufs=2, space="PSUM")

    # ---- x tiles + the DMA program -----------------------------------
    # xtiles[(nm, c, j)] is a [128, 1024] tile holding e-chunks 2j, 2j+1
    # of L-chunk c — one contiguous slice of the host-linearized x*_lin,
    # so each tile is a single DMA with 2KB runs per partition.
    xsrc = {"q": xq, "k": xk, "v": xv}
    xtiles = {}
    for nm in ("k", "q", "v"):
        for c in range(LC):
            for j in range(4):
                xtiles[(nm, c, j)] = xin.tile(
                    [128, 1024], BF16, tag="xin", name=f"x{nm}{c}_{j}"
                )

    def xd(nm, c, j):
        def go(eng):
            col = (c * 4 + j) * 1024
            eng.dma_start(xtiles[(nm, c, j)][:], xsrc[nm][:, col : col + 1024])
        return go

    def wd(wsb, wsrc, g):
        def go(eng):
            sl = slice(g * 1024, (g + 1) * 1024)
            eng.dma_start(wsb[:, sl], wsrc[:, sl])
        return go

    # Only 3 DMA queues exist: sync + scalar (HWDGE) and gpsimd (SWDGE).
    # Measured rates: gpsimd ~175 GB/s (fastest), scalar ~98, sync
    # ~100-135 with a ~6us slow start — so gpsimd carries the critical
    # wave (wk, xk0, xq0) and all k-chunks; scalar carries wq/v0/q1/wo;
    # sync gets second copies and the late chunks.  k-chunks lead
    # v-chunks: scores pace ACT while attn-out may lag behind the sc
    # tile buffer.  "act" preloads the sigmoid ACT table early.  The
    # memset for the warmup tiles runs on vector so the gpsimd queue
    # starts issuing immediately.
    nc.vector.memset(scratch[:], 0.0)

    def act_preload(_):
        nc.scalar.activation(act_warm[:], scratch[:, :8], AF.Sigmoid, scale=SCALE)

    # The scalar ENGINE stream must stay short before its first sigmoid:
    # each issue is paced ~2.9us by the shared HWDGE flow-control ring,
    # and queued sigmoids can't start until the engine drains its issue
    # program.  Scalar therefore issues only the 4 critical weight/xq0
    # transfers; everything else rides gpsimd (fastest) and sync.
    dma_program = {
        "gpsimd": [
            wd(wk_sb, wk, 0), xd("k", 0, 0), xd("k", 0, 1), xd("k", 0, 2),
            xd("k", 0, 3), xd("q", 0, 0),
            xd("k", 1, 0), xd("k", 1, 1), xd("k", 1, 2), xd("k", 2, 0),
            xd("k", 2, 1), xd("k", 2, 2), xd("k", 3, 0), xd("k", 3, 1),
            xd("v", 0, 0), xd("v", 0, 1), xd("v", 0, 2), xd("v", 0, 3),
            xd("k", 3, 2), xd("v", 1, 0), xd("v", 1, 1), xd("q", 1, 2),
            xd("v", 2, 0), xd("v", 2, 1), xd("v", 3, 0), xd("v", 3, 1),
            xd("q", 2, 0), xd("q", 2, 1), xd("q", 3, 0), xd("q", 3, 1),
        ],
        "scalar": [
            wd(wk_sb, wk, 1), wd(wq_sb, wq, 0), wd(wq_sb, wq, 1),
            xd("q", 0, 3), act_preload,
        ],
        "sync": [
            xd("q", 0, 1), xd("q", 0, 2), wd(wv_sb, wv, 0), wd(wv_sb, wv, 1),
            xd("k", 1, 3), xd("k", 2, 3), xd("k", 3, 3),
            xd("q", 1, 0), xd("q", 1, 1), xd("q", 1, 3),
            xd("v", 1, 2), xd("v", 1, 3), xd("v", 2, 2), xd("v", 2, 3),
            wd(wo_sb, wo, 0), wd(wo_sb, wo, 1),
            xd("v", 3, 2), xd("v", 3, 3), xd("q", 2, 2), xd("q", 2, 3),
            xd("q", 3, 2), xd("q", 3, 3),
        ],
    }
    # The two HWDGE queues (scalar, sync) share one 8-deep ring of
    # flow-control semaphores assigned in EMISSION order: emitting one
    # queue's whole program first makes the other queue's issues wait on
    # the first queue's last transfers.  Interleave emission round-robin
    # so the ring distance stays short in time on both queues.
    progs = [
        (getattr(nc, n), list(p)) for n, p in dma_program.items()
    ]
    i = 0
    while any(p for _, p in progs):
        eng, p = progs[i % len(progs)]
        if p:
            p.pop(0)(eng)
        i += 1

    # Warm the PE (HAM clock gate) with scratch matmuls while the first
    # DMAs are in flight.
    wu_ps = ps_sc.tile([128, 1024], F32, tag="ps_sc", name="warmup_ps")
    for i in range(10):
        nc.tensor.matmul(
            wu_ps[:, :512], lhsT=scratch[:, :128], rhs=scratch[:],
            start=(i == 0), stop=(i == 9),
        )

    # ---- projection emitters -----------------------------------------
    def kq_units(nm, c):
        """8 units (one per e-chunk) of the k-/q-projection of L-chunk c."""
        wsb, dst = (wk_sb, kT_sb) if nm == "k" else (wq_sb, qT_sb)
        acc = []

        def unit(e):
            if e == 0:
                acc.extend(
                    ps_proj.tile([128, 512], F32, tag="ps_proj", name=f"{nm}{c}_{mt}")
                    for mt in range(2)
                )
            xt = xtiles[(nm, c, e // 2)][:, (e % 2) * 512 : (e % 2) * 512 + 512]
            for mt in range(2):
                nc.tensor.matmul(
                    acc[mt][:],
                    lhsT=wsb[:, e * DH + mt * 128 : e * DH + (mt + 1) * 128],
                    rhs=xt,
                    start=(e == 0),
                    stop=(e == EC - 1),
                )
            if e == EC - 1:
                for mt in range(2):
                    nc.vector.tensor_copy(
                        dst[:, mt * L + c * 512 : mt * L + (c + 1) * 512],
                        acc[mt][:],
                    )

        for e in range(EC):
            yield lambda e=e: unit(e)

    def v_units(c):
        """8 units ((st4, e-half)) of the v-projection of L-chunk c."""
        for st4 in range(4):
            box = {}

            def unit(st4, eh, box):
                st = c * 4 + st4
                if eh == 0:
                    box["acc"] = ps_proj.tile(
                        [128, DH], F32, tag="ps_proj", name=f"vacc{st}"
                    )
                for e in range(eh * 4, eh * 4 + 4):
                    nc.tensor.matmul(
                        box["acc"][:],
                        lhsT=xtiles[("v", c, e // 2)][
                            :, (e % 2) * 512 + st4 * 128 : (e % 2) * 512 + (st4 + 1) * 128
                        ],
                        rhs=wv_sb[:, e * DH : (e + 1) * DH],
                        start=(e == 0),
                        stop=(e == EC - 1),
                    )
                if eh == 1:
                    nc.vector.tensor_copy(v_sb[:, st * DH : (st + 1) * DH], box["acc"][:])

            for eh in range(2):
                yield lambda st4=st4, eh=eh, box=box: unit(st4, eh, box)

    def outproj_units(lc):
        """4 units (one per l-tile): out[lg:lg+128, :] = oT.T @ wo, cast
        to bf16, one 256KB row-contiguous DMA store.  The last l-chunk
        also stores via the scalar queue — ACT is done by then and three
        queues shorten the tail drain."""
        engs = (
            [nc.sync, nc.gpsimd, nc.scalar, nc.sync]
            if lc == LC - 1
            else [nc.sync, nc.gpsimd, nc.sync, nc.gpsimd]
        )
        last = lc == LC - 1
        for lt in range(4):
            def unit(lt=lt):
                lg = lc * 512 + lt * 128
                ot = ou_pool.tile([128, E], BF16, tag="ou", name=f"ot{lc}_{lt}")
                for ec in range(2):
                    # the tail l-chunk spreads psum across ps_proj+ps_sc
                    # (idle by then) and casts on vector+gpsimd so its 16
                    # matmuls run back-to-back instead of ping-ponging on
                    # two slots behind each cast
                    pool = ps_sc if (last and ec == 1) else ps_proj
                    ps = pool.tile(
                        [128, 512], F32, tag=pool is ps_sc and "ps_sc" or "ps_proj",
                        name=f"ops{lc}_{lt}_{ec}",
                    )
                    for m in range(2):
                        nc.tensor.matmul(
                            ps[:],
                            lhsT=oT_sb[:, m * L + lg : m * L + lg + 128],
                            rhs=wo_sb[:, m * E + ec * 512 : m * E + (ec + 1) * 512],
                            start=(m == 0),
                            stop=(m == 1),
                        )
                    # gpsimd can't read PSUM; scalar (ACT) can and has
                    # finished its sigmoids by the tail
                    if last and (lt + ec) % 2:
                        nc.scalar.copy(ot[:, ec * 512 : (ec + 1) * 512], ps[:])
                    else:
                        nc.vector.tensor_copy(ot[:, ec * 512 : (ec + 1) * 512], ps[:])
                engs[lt].dma_start(out[lg : lg + 128, :], ot[:])
            yield unit

    # ---- attention emitters ------------------------------------------
    sc_tiles = {}

    def emit_scores(lc, st):
        for pair in range(2):
            ps = ps_sc.tile([128, 1024], F32, tag="ps_sc", name=f"scps{lc}_{st}_{pair}")
            for sub in range(2):
                nc.tensor.matmul(
                    ps[:, sub * 512 : (sub + 1) * 512],
                    lhsT=kT_sb[
                        sub * 64 : (sub + 1) * 64,
                        pair * L + st * 128 : pair * L + (st + 1) * 128,
                    ],
                    rhs=qT_sb[
                        sub * 64 : (sub + 1) * 64,
                        pair * L + lc * 512 : pair * L + (lc + 1) * 512,
                    ],
                    start=True,
                    stop=True,
                    tile_position=(sub * 64, 0),
                )
            sc = sc_pool.tile([128, 1024], BF16, tag="sc", name=f"sc{lc}_{st}_{pair}")
            nc.scalar.activation(sc[:], ps[:], AF.Sigmoid, scale=SCALE)
            sc_tiles[(st, pair)] = sc

    def emit_attnout(lc, st, o_acc):
        for pair in range(2):
            for sub in range(2):
                h = pair * 2 + sub
                nc.tensor.matmul(
                    o_acc[pair][sub * 64 : (sub + 1) * 64, :],
                    lhsT=v_sb[:, st * DH + h * 64 : st * DH + (h + 1) * 64],
                    rhs=sc_tiles[(st, pair)][:, sub * 512 : (sub + 1) * 512],
                    start=(st == 0),
                    stop=(st == ST - 1),
                    tile_position=(0, sub * 64),
                    # Sim's psum-group bookkeeping mis-addresses
                    # partition-offset groups; has_written is per-element
                    # on HW and the two halves are disjoint.
                    skip_group_check=True,
                )

    filler = []          # queue of pending closures (lc1-3 only)

    def pop_filler(n):
        for _ in range(min(n, len(filler))):
            filler.pop(0)()

    def flush_filler():
        while filler:
            filler.pop(0)()

    # ---- lc0: fully explicit schedule --------------------------------
    # k0/q0 inline (their DMAs land first); per s-group the PE queue is
    # [scores x4 | k(next) | v(sg-1) | attn-out(sg-1) x4]: scores (which
    # pace ACT) are never queued behind arrival-gated v-projections —
    # attn-out runs one s-group behind, inside the sc tile buffer's
    # elastic window, and catches up.
    for u in kq_units("k", 0):
        u()
    for u in kq_units("q", 0):
        u()

    o_acc0 = [
        ps_o.tile([128, 512], F32, tag="ps_o", name=f"oacc0_{p}") for p in range(2)
    ]
    for sg in range(4):
        for st4 in range(4):
            emit_scores(0, sg * 4 + st4)
        if sg < 3:
            for u in kq_units("k", sg + 1):
                u()
        else:
            for u in kq_units("q", 1):
                u()
        if sg >= 1:
            for u in v_units(sg - 1):
                u()
            for st in range((sg - 1) * 4, sg * 4):
                emit_attnout(0, st, o_acc0)

    # ---- lc1 prefix: drain lc0's deferred tail -----------------------
    # v3 + attn-out st12-15 + the lc0 oT copy interleave with lc1's
    # first scores so ACT rolls straight across the l-chunk boundary.
    for u in v_units(3):
        u()
    o_acc1 = [
        ps_o.tile([128, 512], F32, tag="ps_o", name="oacc1_%d" % p) for p in range(2)
    ]
    emit_scores(1, 0)
    emit_scores(1, 1)
    for st in range(12, 16):
        emit_attnout(0, st, o_acc0)
    for pair in range(2):
        nc.vector.tensor_copy(
            oT_sb[:, pair * L + 0 * 512 : pair * L + 1 * 512], o_acc0[pair][:]
        )

    # ---- lc1-3: ACT-bound steady state --------------------------------
    # filler per lc: q(lc+1) projection then outproj(lc-1); popped only
    # from st>=4 so outproj never queues ahead of a not-yet-emitted oT
    # copy or stalls the first scores of the l-chunk.
    filler.extend(kq_units("q", 2))
    filler.extend(outproj_units(0))
    for lc in range(1, LC):
        if lc == 1:
            o_acc = o_acc1
        else:
            o_acc = [
                ps_o.tile([128, 512], F32, tag="ps_o", name=f"oacc{lc}_{p}")
                for p in range(2)
            ]
        for st in range(16):
            if not (lc == 1 and st < 2):
                emit_scores(lc, st)
            emit_attnout(lc, st, o_acc)
            if st >= 4:
                pop_filler(1)
        flush_filler()
        for pair in range(2):
            nc.vector.tensor_copy(
                oT_sb[:, pair * L + lc * 512 : pair * L + (lc + 1) * 512],
                o_acc[pair][:],
            )
        if lc == 1:
            filler.extend(kq_units("q", 3))
            filler.extend(outproj_units(1))
        elif lc == 2:
            filler.extend(outproj_units(2))
        else:
            filler.extend(outproj_units(3))
    flush_filler()

    # release pools in reverse allocation (stack) order
    for pool in (ps_o, ps_sc, ps_proj, xin, ou_pool, sc_pool, persist):
        pool.release()


_NC_CACHE = None


def _get_nc():
    global _NC_CACHE
    if _NC_CACHE is None:
        _NC_CACHE = build_nc()
    return _NC_CACHE


def _lin_x(xT):
    """[1024, 2048] -> [128, 16384] with col (c*8+e)*512+l = xT[e*128+p, c*512+l]."""
    return np.ascontiguousarray(
        xT.reshape(8, 128, 4, 512).transpose(1, 2, 0, 3).reshape(128, 16384)
    )


def _prep_in_maps(query, key, value, Wq, Wk, Wv, Wo):
    B = query.shape[0]
    bf = ml_dtypes.bfloat16
    xlin = {}
    for b in range(B):
        xlin[("q", b)] = _lin_x(query[b].T.astype(bf))
        xlin[("k", b)] = _lin_x(key[b].T.astype(bf))
        xlin[("v", b)] = _lin_x(value[b].T.astype(bf))
    in_maps = []
    for c in range(N_CORES):
        b, g = c // 4, c % 4
        hs = slice(g * DH, (g + 1) * DH)

        def wlin(W):
            wT = np.ascontiguousarray(W[hs, :].T).astype(bf)   # [1024, 256]
            return np.ascontiguousarray(
                wT.reshape(8, 128, 256).transpose(1, 0, 2).reshape(128, 2048)
            )

        woT = np.ascontiguousarray(Wo[:, hs].T).astype(bf)     # [256, 1024]
        wo_l = np.ascontiguousarray(
            woT.reshape(2, 128, 1024).transpose(1, 0, 2).reshape(128, 2048)
        )
        in_maps.append(
            {
                "xq_lin": xlin[("q", b)],
                "xk_lin": xlin[("k", b)],
                "xv_lin": xlin[("v", b)],
                "wq_lin": wlin(Wq),
                "wk_lin": wlin(Wk),
                "wv_lin": wlin(Wv),
                "wo_lin": wo_l,
            }
        )
    return in_maps


LAST_RESULTS = None


def run_sharded(query, key, value, Wq, Wk, Wv, Wo, trace=False, tmpdir=None):
    global LAST_RESULTS
    if trace:
        # Shim the missing antenv.axon_hooks so NTFF tracing works under axon.
        import sys
        import types

        try:
            import antenv.axon_hooks  # noqa: F401
        except ImportError:
            from trn_agent_boot.trn_boot import _ntff_profile_via_ctypes

            _mod = types.ModuleType("antenv.axon_hooks")
            _hook = _ntff_profile_via_ctypes("/opt/axon/libaxon_pjrt.so")
            _mod.get_axon_ntff_profile_hook = lambda: _hook
            sys.modules["antenv.axon_hooks"] = _mod
        bass_utils.upload_artifacts = lambda tmpdir: tmpdir

    nc = _get_nc()
    in_maps = _prep_in_maps(query, key, value, Wq, Wk, Wv, Wo)
    res = bass_utils.run_bass_kernel_spmd(
        nc, in_maps, core_ids=list(range(N_CORES)), trace=trace, tmpdir=tmpdir
    )
    LAST_RESULTS = res
    B = query.shape[0]
    full = np.zeros((B, L, E), dtype=np.float32)
    for c in range(N_CORES):
        full[c // 4] += np.asarray(res.results[c]["out"], dtype=np.float32)
    return full


def kernel(query, key, value, Wq, Wk, Wv, Wo):
    return run_sharded(query, key, value, Wq, Wk, Wv, Wo, trace=False)


# revision 14
# speedup vs baseline: 1.0408x; 1.0034x over previous
"""Chunked sigmoid MHA on 8 Trainium2 NeuronCores (Bass/Tile).

Problem: out = (sigmoid(scale * (x_q Wq^T)(x_k Wk^T)^T) @ (x_v Wv^T)) @ Wo^T
with B=2, L=S=2048, E=1024, H=16, D=64.

Sharding: (batch, head-group) — core c handles batch b=c//4 and heads
[4g, 4g+4) with g=c%4.  Each core computes its 4 heads' Q/K/V projections
(column slices of Wq/Wk/Wv), full sigmoid attention for those heads, and a
partial output projection (row slice of Wo^T); the host sums the 4 partial
outputs per batch.

The kernel is ACT-bound: 128 sigmoid instructions of [128, 1024] are
~143.7 us of scalar-engine time, while true PE stream time is ~117 us
(score/attn-out matmul pairs run concurrently via tile_position row/col
tiling).  The design therefore optimizes ACT occupancy:
  - all host tensors are pre-linearized so each SBUF tile is one
    contiguous DMA (2KB runs per partition), spread over all 5 engine
    queues; critical wk/wq/xk0/xq0 land first -> first sigmoid ~16us
  - the sigmoid ACT table is preloaded at t~0
  - lc0 interleaves per-s-group [scores | k-proj(next) | v-proj | attn-out]
    so scores (which feed ACT) are never queued behind arrival-gated
    projection work; q projections and output projections are deferred
    to lc1-3 where the PE is underloaded
  - output is stored bf16 (halves store traffic), host accumulates f32
"""

import ml_dtypes
import numpy as np

import concourse.bass as bass
import concourse.mybir as mybir
import concourse.tile as tile
from concourse import bass_utils
from concourse.vector_clock import ScopedClock

F32 = mybir.dt.float32
BF16 = mybir.dt.bfloat16
AF = mybir.ActivationFunctionType

E = 1024          # embed dim
L = 2048          # sequence length (queries == keys)
DH = 256          # per-core projection dim (4 heads x 64)
EC = E // 128     # 8 E-chunks of 128
LC = L // 512     # 4 L-chunks of 512
ST = L // 128     # 16 S-tiles of 128
SCALE = 64 ** -0.5  # 0.125, applied inside the sigmoid activation

N_CORES = 8


class SplitDrainTileContext(tile.TileContext):
    """This walrus build rejects >1 sync wait on the SP CTRL (Drain)
    instruction, and Tile's end-of-kernel drain waits on every used proc.
    Split the waits across a chain of single-wait drains."""

    DRAIN_WAIT_CAP = 1

    def _drain_and_barrier(self, tick_clock, wait_clock):
        nc = self.nc
        drain_inst = nc.sync.drain()
        wait_clock.add_sem_waits(
            drain_inst.ins, ScopedClock({None: tick_clock.global_clock})
        )
        si = drain_inst.ins.sync_info
        waits = list(si.on_wait) if si is not None else []
        if len(waits) > self.DRAIN_WAIT_CAP:
            si.on_wait = waits[: self.DRAIN_WAIT_CAP]
            for i in range(self.DRAIN_WAIT_CAP, len(waits), self.DRAIN_WAIT_CAP):
                extra = nc.sync.drain()
                esi = extra.ins.sync_info
                if esi is None:
                    esi = mybir.SyncInfo(on_wait=[], on_update=[])
                esi.on_wait = waits[i : i + self.DRAIN_WAIT_CAP]
                extra.ins.sync_info = esi
        nc.all_engine_barrier()
        assert self.sems is not None
        popped = nc._tile_sem_poison_stack.pop()
        assert popped is self._sem_poison
        nc.clear_and_free_semaphores(list(self.sems.allocated().values()))
        nc.all_engine_barrier()


def build_nc() -> bass.Bass:
    nc = bass.Bass("TRN2", target_bir_lowering=False, debug=False)

    # Host-linearized layouts (see _prep_in_maps):
    #   x*_lin [128, 16384]: col (c*8+e)*512 + l  =  xT[e*128+p, c*512+l]
    #   w*_lin [128, 2048]:  col e*256 + m        =  w*T[e*128+p, m]
    #   wo_lin [128, 2048]:  col m*1024 + eo      =  wo[m*128+p, eo]
    xq = nc.dram_tensor("xq_lin", [128, 16384], BF16, kind="ExternalInput").ap()
    xk = nc.dram_tensor("xk_lin", [128, 16384], BF16, kind="ExternalInput").ap()
    xv = nc.dram_tensor("xv_lin", [128, 16384], BF16, kind="ExternalInput").ap()
    wq = nc.dram_tensor("wq_lin", [128, 2048], BF16, kind="ExternalInput").ap()
    wk = nc.dram_tensor("wk_lin", [128, 2048], BF16, kind="ExternalInput").ap()
    wv = nc.dram_tensor("wv_lin", [128, 2048], BF16, kind="ExternalInput").ap()
    wo = nc.dram_tensor("wo_lin", [128, 2048], BF16, kind="ExternalInput").ap()
    out = nc.dram_tensor("out", [L, E], BF16, kind="ExternalOutput").ap()

    with SplitDrainTileContext(nc) as tc:
        body(tc, xq, xk, xv, wq, wk, wv, wo, out)
    _split_waits(nc)
    return nc


def _split_waits(nc, cap=1):
    """This walrus build rejects instructions carrying more than one sync
    wait.  Hoist excess waits onto same-engine NoOps inserted immediately
    before the instruction (engine program order enforces them first)."""
    ctr = 0
    for f in nc.m.functions:
        for bb in f.blocks:
            new = []
            for inst in bb.instructions:
                si = inst.sync_info
                waits = list(si.on_wait) if si is not None else []
                if len(waits) > cap:
                    for i in range(cap, len(waits), cap):
                        ctr += 1
                        nop = mybir.InstNoOp(name=f"I-waitnop-{ctr}")
                        nop.engine = inst.engine
                        nop.sync_info = mybir.SyncInfo(
                            on_wait=waits[i : i + cap], on_update=[]
                        )
                        nc.register_instruction(nop)
                        new.append(nop)
                    si.on_wait = waits[:cap]
                new.append(inst)
            bb.instructions = new
    return ctr


def body(tc, xq, xk, xv, wq, wk, wv, wo, out):
    nc = tc.nc

    # ---- persistent SBUF tensors -------------------------------------
    persist = tc.alloc_tile_pool(name="persist", bufs=1)

    def ptile(name, shape):
        return persist.tile(shape, BF16, tag=name, name=name)

    # weights, E-chunk-major: w*_sb[:, e*256+m] = w*T[e*128+p, m]
    wq_sb = ptile("wq_sb", [128, 2048])
    wk_sb = ptile("wk_sb", [128, 2048])
    wv_sb = ptile("wv_sb", [128, 2048])
    # wo, m-chunk-major: wo_sb[:, m*1024+e] = wo[m*128+p, e]
    wo_sb = ptile("wo_sb", [128, 2 * E])
    # projected tensors: qT/kT [dh, L] stored Mt-major; v natural [S, dh]
    # stored St-major; oT [dh, L] stored m-chunk-major
    qT_sb = ptile("qT_sb", [128, 2 * L])
    kT_sb = ptile("kT_sb", [128, 2 * L])
    v_sb = persist.tile([128, ST * DH], BF16, tag="v_sb", name="v_sb")
    oT_sb = ptile("oT_sb", [128, 2 * L])
    scratch = persist.tile([128, 512], BF16, tag="scratch", name="scratch")
    act_warm = persist.tile([128, 8], BF16, tag="act_warm", name="act_warm")

    # sc bufs=24 gives ACT/attn-out a 12-step elastic window so late v
    # arrivals (v-chunks are the lowest-priority DMAs) never stall the
    # sigmoid chain
    sc_pool = tc.alloc_tile_pool(name="sc", bufs=24)
    ou_pool = tc.alloc_tile_pool(name="ou", bufs=3)
    xin = tc.alloc_tile_pool(name="xin", bufs=48)
    ps_proj = tc.alloc_tile_pool(name="ps_proj", bufs=2, space="PSUM")
    ps_sc = tc.alloc_tile_pool(name="ps_sc", bufs=2, space="PSUM")
    ps_o = tc.alloc_tile_pool(name="ps_o", bufs=2, space="PSUM")

    # ---- x tiles + the DMA program -----------------------------------
    # xtiles[(nm, c, j)] is a [128, 1024] tile holding e-chunks 2j, 2j+1
    # of L-chunk c — one contiguous slice of the host-linearized x*_lin,
    # so each tile is a single DMA with 2KB runs per partition.
    xsrc = {"q": xq, "k": xk, "v": xv}
    xtiles = {}
    for nm in ("k", "q", "v"):
        for c in range(LC):
            for j in range(4):
                xtiles[(nm, c, j)] = xin.tile(
                    [128, 1024], BF16, tag="xin", name=f"x{nm}{c}_{j}"
                )

    def xd(nm, c, j):
        def go(eng):
            col = (c * 4 + j) * 1024
            eng.dma_start(xtiles[(nm, c, j)][:], xsrc[nm][:, col : col + 1024])
        return go

    def wd(wsb, wsrc, g):
        def go(eng):
            sl = slice(g * 1024, (g + 1) * 1024)
            eng.dma_start(wsb[:, sl], wsrc[:, sl])
        return go

    # Only 3 DMA queues exist: sync + scalar (HWDGE) and gpsimd (SWDGE).
    # Measured rates: gpsimd ~175 GB/s (fastest), scalar ~98, sync
    # ~100-135 with a ~6us slow start — so gpsimd carries the critical
    # wave (wk, xk0, xq0) and all k-chunks; scalar carries wq/v0/q1/wo;
    # sync gets second copies and the late chunks.  k-chunks lead
    # v-chunks: scores pace ACT while attn-out may lag behind the sc
    # tile buffer.  "act" preloads the sigmoid ACT table early.  The
    # memset for the warmup tiles runs on vector so the gpsimd queue
    # starts issuing immediately.
    nc.vector.memset(scratch[:], 0.0)

    def act_preload(_):
        nc.scalar.activation(act_warm[:], scratch[:, :8], AF.Sigmoid, scale=SCALE)

    # The scalar ENGINE stream must stay short before its first sigmoid:
    # each issue is paced ~2.9us by the shared HWDGE flow-control ring,
    # and queued sigmoids can't start until the engine drains its issue
    # program.  Scalar therefore issues only the 4 critical weight/xq0
    # transfers; everything else rides gpsimd (fastest) and sync.
    dma_program = {
        "gpsimd": [
            wd(wk_sb, wk, 0), xd("k", 0, 0), xd("k", 0, 1), xd("k", 0, 2),
            xd("k", 0, 3), xd("q", 0, 0),
            xd("k", 1, 0), xd("k", 1, 1), xd("k", 1, 2), xd("k", 2, 0),
            xd("k", 2, 1), xd("k", 2, 2), xd("k", 3, 0), xd("k", 3, 1),
            xd("v", 0, 0), xd("v", 0, 1), xd("v", 0, 2), xd("v", 0, 3),
            xd("k", 3, 2), xd("v", 1, 0), xd("v", 1, 1), xd("q", 1, 2),
            xd("v", 2, 0), xd("v", 2, 1), xd("v", 3, 0), xd("v", 3, 1),
            xd("q", 2, 0), xd("q", 2, 1), xd("q", 3, 0), xd("q", 3, 1),
        ],
        "scalar": [
            wd(wk_sb, wk, 1), wd(wq_sb, wq, 0), wd(wq_sb, wq, 1),
            xd("q", 0, 3), act_preload,
        ],
        "sync": [
            xd("q", 0, 1), xd("q", 0, 2), wd(wv_sb, wv, 0), wd(wv_sb, wv, 1),
            xd("k", 1, 3), xd("k", 2, 3), xd("k", 3, 3),
            xd("q", 1, 0), xd("q", 1, 1), xd("q", 1, 3),
            xd("v", 1, 2), xd("v", 1, 3), xd("v", 2, 2), xd("v", 2, 3),
            wd(wo_sb, wo, 0), wd(wo_sb, wo, 1),
            xd("v", 3, 2), xd("v", 3, 3), xd("q", 2, 2), xd("q", 2, 3),
            xd("q", 3, 2), xd("q", 3, 3),
        ],
    }
    # The two HWDGE queues (scalar, sync) share one 8-deep ring of
    # flow-control semaphores assigned in EMISSION order: emitting one
    # queue's whole program first makes the other queue's issues wait on
    # the first queue's last transfers.  Interleave emission round-robin
    # so the ring distance stays short in time on both queues.
    progs = [
        (getattr(nc, n), list(p)) for n, p in dma_program.items()
    ]
    i = 0
    while any(p for _, p in progs):
        eng, p = progs[i % len(progs)]
        if p:
            p.pop(0)(eng)
        i += 1

    # Warm the PE (HAM clock gate) with scratch matmuls while the first
    # DMAs are in flight.
    wu_ps = ps_sc.tile([128, 1024], F32, tag="ps_sc", name="warmup_ps")
    for i in range(10):
        nc.tensor.matmul(
            wu_ps[:, :512], lhsT=scratch[:, :128], rhs=scratch[:],
            start=(i == 0), stop=(i == 9),
        )

    # ---- projection emitters -----------------------------------------
    def kq_units(nm, c):
        """8 units (one per e-chunk) of the k-/q-projection of L-chunk c."""
        wsb, dst = (wk_sb, kT_sb) if nm == "k" else (wq_sb, qT_sb)
        acc = []

        def unit(e):
            if e == 0:
                acc.extend(
                    ps_proj.tile([128, 512], F32, tag="ps_proj", name=f"{nm}{c}_{mt}")
                    for mt in range(2)
                )
            xt = xtiles[(nm, c, e // 2)][:, (e % 2) * 512 : (e % 2) * 512 + 512]
            for mt in range(2):
                nc.tensor.matmul(
                    acc[mt][:],
                    lhsT=wsb[:, e * DH + mt * 128 : e * DH + (mt + 1) * 128],
                    rhs=xt,
                    start=(e == 0),
                    stop=(e == EC - 1),
                )
            if e == EC - 1:
                for mt in range(2):
                    nc.vector.tensor_copy(
                        dst[:, mt * L + c * 512 : mt * L + (c + 1) * 512],
                        acc[mt][:],
                    )

        for e in range(EC):
            yield lambda e=e: unit(e)

    def v_units(c):
        """8 units ((st4, e-half)) of the v-projection of L-chunk c."""
        for st4 in range(4):
            box = {}

            def unit(st4, eh, box):
                st = c * 4 + st4
                if eh == 0:
                    box["acc"] = ps_proj.tile(
                        [128, DH], F32, tag="ps_proj", name=f"vacc{st}"
                    )
                for e in range(eh * 4, eh * 4 + 4):
                    nc.tensor.matmul(
                        box["acc"][:],
                        lhsT=xtiles[("v", c, e // 2)][
                            :, (e % 2) * 512 + st4 * 128 : (e % 2) * 512 + (st4 + 1) * 128
                        ],
                        rhs=wv_sb[:, e * DH : (e + 1) * DH],
                        start=(e == 0),
                        stop=(e == EC - 1),
                    )
                if eh == 1:
                    nc.vector.tensor_copy(v_sb[:, st * DH : (st + 1) * DH], box["acc"][:])

            for eh in range(2):
                yield lambda st4=st4, eh=eh, box=box: unit(st4, eh, box)

    def outproj_units(lc):
        """4 units (one per l-tile): out[lg:lg+128, :] = oT.T @ wo, cast
        to bf16, one 256KB row-contiguous DMA store.  The last l-chunk
        also stores via the scalar queue — ACT is done by then and three
        queues shorten the tail drain."""
        engs = (
            [nc.sync, nc.gpsimd, nc.scalar, nc.sync]
            if lc == LC - 1
            else [nc.sync, nc.gpsimd, nc.sync, nc.gpsimd]
        )
        last = lc == LC - 1
        for lt in range(4):
            def unit(lt=lt):
                lg = lc * 512 + lt * 128
                ot = ou_pool.tile([128, E], BF16, tag="ou", name=f"ot{lc}_{lt}")
                for ec in range(2):
                    # the tail l-chunk spreads psum across ps_proj+ps_sc
                    # (idle by then) and casts on vector+gpsimd so its 16
                    # matmuls run back-to-back instead of ping-ponging on
                    # two slots behind each cast
                    pool = ps_sc if (last and ec == 1) else ps_proj
                    ps = pool.tile(
                        [128, 512], F32, tag=pool is ps_sc and "ps_sc" or "ps_proj",
                        name=f"ops{lc}_{lt}_{ec}",
                    )
                    for m in range(2):
                        nc.tensor.matmul(
                            ps[:],
                            lhsT=oT_sb[:, m * L + lg : m * L + lg + 128],
                            rhs=wo_sb[:, m * E + ec * 512 : m * E + (ec + 1) * 512],
                            start=(m == 0),
                            stop=(m == 1),
                        )
                    # gpsimd can't read PSUM; scalar (ACT) can and has
                    # finished its sigmoids by the tail
                    if last and (lt + ec) % 2:
                        nc.scalar.copy(ot[:, ec * 512 : (ec + 1) * 512], ps[:])
                    else:
                        nc.vector.tensor_copy(ot[:, ec * 512 : (ec + 1) * 512], ps[:])
                engs[lt].dma_start(out[lg : lg + 128, :], ot[:])
            yield unit

    # ---- attention emitters ------------------------------------------
    sc_tiles = {}

    def emit_scores(lc, st):
        for pair in range(2):
            ps = ps_sc.tile([128, 1024], F32, tag="ps_sc", name=f"scps{lc}_{st}_{pair}")
            for sub in range(2):
                nc.tensor.matmul(
                    ps[:, sub * 512 : (sub + 1) * 512],
                    lhsT=kT_sb[
                        sub * 64 : (sub + 1) * 64,
                        pair * L + st * 128 : pair * L + (st + 1) * 128,
                    ],
                    rhs=qT_sb[
                        sub * 64 : (sub + 1) * 64,
                        pair * L + lc * 512 : pair * L + (lc + 1) * 512,
                    ],
                    start=True,
                    stop=True,
                    tile_position=(sub * 64, 0),
                )
            sc = sc_pool.tile([128, 1024], BF16, tag="sc", name=f"sc{lc}_{st}_{pair}")
            nc.scalar.activation(sc[:], ps[:], AF.Sigmoid, scale=SCALE)
            sc_tiles[(st, pair)] = sc

    def emit_attnout(lc, st, o_acc):
        for pair in range(2):
            for sub in range(2):
                h = pair * 2 + sub
                nc.tensor.matmul(
                    o_acc[pair][sub * 64 : (sub + 1) * 64, :],
                    lhsT=v_sb[:, st * DH + h * 64 : st * DH + (h + 1) * 64],
                    rhs=sc_tiles[(st, pair)][:, sub * 512 : (sub + 1) * 512],
                    start=(st == 0),
                    stop=(st == ST - 1),
                    tile_position=(0, sub * 64),
                    # Sim's psum-group bookkeeping mis-addresses
                    # partition-offset groups; has_written is per-element
                    # on HW and the two halves are disjoint.
                    skip_group_check=True,
                )

    filler = []          # queue of pending closures (lc1-3 only)

    def pop_filler(n):
        for _ in range(min(n, len(filler))):
            filler.pop(0)()

    def flush_filler():
        while filler:
            filler.pop(0)()

    # ---- lc0: fully explicit schedule --------------------------------
    # k0/q0 inline (their DMAs land first); per s-group the PE queue is
    # [scores x4 | k(next) | v(sg-1) | attn-out(sg-1) x4]: scores (which
    # pace ACT) are never queued behind arrival-gated v-projections —
    # attn-out runs one s-group behind, inside the sc tile buffer's
    # elastic window, and catches up.
    for u in kq_units("k", 0):
        u()
    for u in kq_units("q", 0):
        u()

    o_acc0 = [
        ps_o.tile([128, 512], F32, tag="ps_o", name=f"oacc0_{p}") for p in range(2)
    ]
    for sg in range(4):
        for st4 in range(4):
            emit_scores(0, sg * 4 + st4)
        if sg < 3:
            for u in kq_units("k", sg + 1):
                u()
        else:
            for u in kq_units("q", 1):
                u()
        if 1 <= sg <= 2:
            for u in v_units(sg - 1):
                u()
            for st in range((sg - 1) * 4, sg * 4):
                emit_attnout(0, st, o_acc0)

    # ---- lc1 prefix: drain lc0's deferred tail -----------------------
    # lc0's window is PE-oversubscribed by ~7us, so v2/v3 + attn-out
    # st8-15 + the lc0 oT copy move into lc1's slack, interleaved with
    # lc1's first scores so ACT rolls straight across the boundary.
    # lc1's own attn-outs then lag behind the oT-copy WAR on the ps_o
    # ring, inside the sc buffer's elastic window.
    o_acc1 = [
        ps_o.tile([128, 512], F32, tag="ps_o", name="oacc1_%d" % p) for p in range(2)
    ]
    emit_scores(1, 0)
    for u in v_units(2):
        u()
    for st in range(8, 12):
        emit_attnout(0, st, o_acc0)
    emit_scores(1, 1)
    for u in v_units(3):
        u()
    for st in range(12, 16):
        emit_attnout(0, st, o_acc0)
    for pair in range(2):
        nc.vector.tensor_copy(
            oT_sb[:, pair * L + 0 * 512 : pair * L + 1 * 512], o_acc0[pair][:]
        )

    # ---- lc1-3: ACT-bound steady state --------------------------------
    # filler per lc: q(lc+1) projection then outproj(lc-1); popped only
    # from st>=4 so outproj never queues ahead of a not-yet-emitted oT
    # copy or stalls the first scores of the l-chunk.
    filler.extend(kq_units("q", 2))
    filler.extend(outproj_units(0))
    for lc in range(1, LC):
        if lc == 1:
            o_acc = o_acc1
        else:
            o_acc = [
                ps_o.tile([128, 512], F32, tag="ps_o", name=f"oacc{lc}_{p}")
                for p in range(2)
            ]
        for st in range(16):
            if not (lc == 1 and st < 2):
                emit_scores(lc, st)
            emit_attnout(lc, st, o_acc)
            if st >= 4:
                pop_filler(1)
        flush_filler()
        for pair in range(2):
            nc.vector.tensor_copy(
                oT_sb[:, pair * L + lc * 512 : pair * L + (lc + 1) * 512],
                o_acc[pair][:],
            )
        if lc == 1:
            filler.extend(kq_units("q", 3))
            filler.extend(outproj_units(1))
        elif lc == 2:
            filler.extend(outproj_units(2))
        else:
            filler.extend(outproj_units(3))
    flush_filler()

    # release pools in reverse allocation (stack) order
    for pool in (ps_o, ps_sc, ps_proj, xin, ou_pool, sc_pool, persist):
        pool.release()


_NC_CACHE = None


def _get_nc():
    global _NC_CACHE
    if _NC_CACHE is None:
        _NC_CACHE = build_nc()
    return _NC_CACHE


def _lin_x(xT):
    """[1024, 2048] -> [128, 16384] with col (c*8+e)*512+l = xT[e*128+p, c*512+l]."""
    return np.ascontiguousarray(
        xT.reshape(8, 128, 4, 512).transpose(1, 2, 0, 3).reshape(128, 16384)
    )


def _prep_in_maps(query, key, value, Wq, Wk, Wv, Wo):
    B = query.shape[0]
    bf = ml_dtypes.bfloat16
    xlin = {}
    for b in range(B):
        xlin[("q", b)] = _lin_x(query[b].T.astype(bf))
        xlin[("k", b)] = _lin_x(key[b].T.astype(bf))
        xlin[("v", b)] = _lin_x(value[b].T.astype(bf))
    in_maps = []
    for c in range(N_CORES):
        b, g = c // 4, c % 4
        hs = slice(g * DH, (g + 1) * DH)

        def wlin(W):
            wT = np.ascontiguousarray(W[hs, :].T).astype(bf)   # [1024, 256]
            return np.ascontiguousarray(
                wT.reshape(8, 128, 256).transpose(1, 0, 2).reshape(128, 2048)
            )

        woT = np.ascontiguousarray(Wo[:, hs].T).astype(bf)     # [256, 1024]
        wo_l = np.ascontiguousarray(
            woT.reshape(2, 128, 1024).transpose(1, 0, 2).reshape(128, 2048)
        )
        in_maps.append(
            {
                "xq_lin": xlin[("q", b)],
                "xk_lin": xlin[("k", b)],
                "xv_lin": xlin[("v", b)],
                "wq_lin": wlin(Wq),
                "wk_lin": wlin(Wk),
                "wv_lin": wlin(Wv),
                "wo_lin": wo_l,
            }
        )
    return in_maps


LAST_RESULTS = None


def run_sharded(query, key, value, Wq, Wk, Wv, Wo, trace=False, tmpdir=None):
    global LAST_RESULTS
    if trace:
        # Shim the missing antenv.axon_hooks so NTFF tracing works under axon.
        import sys
        import types

        try:
            import antenv.axon_hooks  # noqa: F401
        except ImportError:
            from trn_agent_boot.trn_boot import _ntff_profile_via_ctypes

            _mod = types.ModuleType("antenv.axon_hooks")
            _hook = _ntff_profile_via_ctypes("/opt/axon/libaxon_pjrt.so")
            _mod.get_axon_ntff_profile_hook = lambda: _hook
            sys.modules["antenv.axon_hooks"] = _mod
        bass_utils.upload_artifacts = lambda tmpdir: tmpdir

    nc = _get_nc()
    in_maps = _prep_in_maps(query, key, value, Wq, Wk, Wv, Wo)
    res = bass_utils.run_bass_kernel_spmd(
        nc, in_maps, core_ids=list(range(N_CORES)), trace=trace, tmpdir=tmpdir
    )
    LAST_RESULTS = res
    B = query.shape[0]
    full = np.zeros((B, L, E), dtype=np.float32)
    for c in range(N_CORES):
        full[c // 4] += np.asarray(res.results[c]["out"], dtype=np.float32)
    return full


def kernel(query, key, value, Wq, Wk, Wv, Wo):
    return run_sharded(query, key, value, Wq, Wk, Wv, Wo, trace=False)
